# revision 1
# baseline (speedup 1.0000x reference)
"""CORDIV stochastic-computing division kernel for Trainium2 (8 NeuronCores).

Recurrence per lane n (T sequential steps, lanes fully independent):
    sr = sr_init[:, n]                       # shift register, depth B
    for t in range(T):
        r  = rng_table[t % B]
        hq = sr[r]
        q[t, n] = dividend[t, n] if divisor[t, n] == 1 else hq
        sr = [q[t, n], sr[0], ..., sr[B-2]]

Unrolled, the shift register disappears:
    q[t] = divisor[t] ? dividend[t] : src_t
    src_t = q[t-1-r_t]          if t-1-r_t >= 0
          = sr_init[r_t - t]    otherwise
and since every stream is bits {0,1}, the select collapses to ONE compare
against a host-packed selector w in {0,1,2}:
    q[t] = is_ge(src_t, w[t]);  w = 1 if dvs=0 (pass hq),
                                    0 if dvs=1,dvd=1 (emit 1),
                                    2 if dvs=1,dvd=0 (emit 0)
The (tiny) gather schedule is resolved on the host from rng_table, so the
device kernel is a static DAG: a single DVE tensor_tensor per step.

Memory-regime optimizations:
  * Every stream is bits: the host folds dividend+divisor into the single
    uint8 selector stream w (loads drop 8x vs f32 pairs).
  * The output is ALSO stored as uint8 (SWDGE bf16 -> u8 cast in the DMA
    datapath) and expanded to f32 on the host: HBM traffic per core is
    ~12.8 MiB (vs 50 MiB naive f32) — a ~36 us DMA floor.
  * Work is spread across all engines so each stays under that floor:
    SP/HWDGE queue does the u8 loads, the scalar engine (ACT) does one
    u8 -> bf16 convert per step pair, DVE runs the 2-op bf16 chain in the
    2x perf mode, and the gpsimd/SWDGE path does the cast-stores.
  * Streams are interleaved on the host into the exact on-chip tile layout
    and loaded two steps at a time; output rows are stored in pairs.
  * This walrus accepts at most ONE sync wait per instruction; extra waits
    are legalized onto preceding same-engine NoOps (_legalize_waits), and
    the structure keeps multi-wait joins rare (q tiles never recycled).

Sharding: lane dimension N split evenly across 8 cores (data parallel,
no communication).
"""

import ml_dtypes
import numpy as np

import concourse.bass as bass
import concourse.mybir as mybir
from concourse.tile import TileContext
from concourse.bass_utils import run_bass_kernel_spmd

N_CORES = 8
P = 128  # SBUF partitions
BF16 = ml_dtypes.bfloat16

_nc_cache: dict = {}
LAST_RESULTS = None  # test harness introspection
REPS = 1  # >1: wrap body in a HW loop (timing harness only; output unchanged)


def _schedule(T, buf_dep, rng_table):
    """Host-side resolution of the shift-register gather into a static DAG.

    Returns (sched, sr_rows): sched[t] = ("q", j) meaning src is quotient row
    j, or ("s", k) meaning src is the k-th entry of sr_rows (a compacted list
    of the sr_init rows actually referenced).
    """
    rng = [int(rng_table[t % buf_dep]) for t in range(T)]
    sched = []
    for t in range(T):
        r = rng[t]
        j = t - 1 - r
        if j >= 0:
            sched.append(("q", j))
        else:
            sched.append(("s", r - t))
    sr_rows = sorted({k for kind, k in sched if kind == "s"})
    row_pos = {k: i for i, k in enumerate(sr_rows)}
    sched = [(kind, k if kind == "q" else row_pos[k]) for kind, k in sched]
    return tuple(sched), sr_rows


def _legalize_waits(nc):
    """Make the emitted BIR digestible by this walrus build.

    1. InstIncSwdgeSem (For_i loop skip/back-edge SWDGE sem adjustment)
       serializes with an empty ISA payload here ("ISA wrong length").
       It is just a contiguous-range semaphore add/sub — rewrite it as
       NoOps carrying equivalent SyncUpdates.
    2. codegen accepts at most ONE sync wait per instruction (any opcode,
       Drain included). Extra waits are hoisted onto preceding same-engine
       NoOps — engines execute their streams in order, so blocking
       semantics are identical.
    """
    n = 0
    mode_map = {"add": "sem-add-imm", "sub": "sem-sub-imm", "wr": "sem-wr-imm"}
    for blk in nc.m.functions[0].blocks:
        new_insts = []
        for inst in blk.instructions:
            if type(inst).__name__ == "InstIncSwdgeSem":
                # 'add' appears only in the loop-skip block (taken when the
                # trip count is <= 0 — never, for the reps timing builds);
                # its waits are all trivially-true >=0. Drop it. 'sub'
                # (back-edge DMASW rewind) becomes per-sem NoOps with
                # sem-sub-imm — the exact pattern Tile's own reset NoOps
                # use, which this walrus encodes fine.
                if inst._mode == "add":
                    continue
                assert inst._mode == "sub", inst._mode
                for i, (val, name) in enumerate(
                    zip(inst._sem_values, inst._sem_names)
                ):
                    if val == 0:
                        continue
                    upd = mybir.SyncUpdate(
                        sync_type="semaphore",
                        id=inst._sem_id_base + i,
                        update_mode="sem-sub-imm",
                        update_value=val,
                        ant_name=name,
                    )
                    new_insts.append(
                        mybir.InstNoOp(
                            name=f"{inst.name}_swdgesem_{n}",
                            engine=inst.engine,
                            ins=[],
                            outs=[],
                            sync_info=mybir.SyncInfo(
                                on_wait=[], on_update=[upd]
                            ),
                        )
                    )
                    n += 1
            else:
                new_insts.append(inst)
        blk.instructions = new_insts
    for blk in nc.m.functions[0].blocks:
        new_insts = []
        for inst in blk.instructions:
            si = inst.sync_info
            waits = list(si.on_wait) if si is not None and si.on_wait is not None else []
            if len(waits) > 1 and inst.opcode != "ISA":
                for w in waits[:-1]:
                    nop = mybir.InstNoOp(
                        name=f"{inst.name}_waitnop_{n}",
                        engine=inst.engine,
                        ins=[],
                        outs=[],
                        sync_info=mybir.SyncInfo(on_wait=[w], on_update=[]),
                    )
                    new_insts.append(nop)
                    n += 1
                inst.sync_info = mybir.SyncInfo(
                    on_wait=[waits[-1]], on_update=list(si.on_update or [])
                )
            new_insts.append(inst)
        blk.instructions = new_insts
    return nc


def _build(T, NS, sched, n_sr, reps=1, legalize=True):
    """Emit the per-core Bass/Tile module. NS = lanes per core."""
    C = NS // P
    bf = mybir.dt.bfloat16
    u8 = mybir.dt.uint8
    nsr = max(n_sr, 1)
    assert T % 2 == 0, T
    nc = bass.Bass()
    # host pre-arranged: bits[u][p][v*2C + s*C + c] for step pair u with
    # s in {divisor, m=dividend*divisor} — each pair-load is one fully
    # contiguous 0.5 MiB 2-D DMA, cast u8 -> bf16 in the SWDGE datapath
    bits = nc.dram_tensor("bits", [T // 2, P, 2 * C], u8, kind="ExternalInput")
    sri = nc.dram_tensor("sr_init", [nsr, NS], u8, kind="ExternalInput")
    out = nc.dram_tensor("quotient", [T, NS], u8, kind="ExternalOutput")

    bits_r = bits[:]
    sri_r = sri[:].rearrange("k (p c) -> p k c", p=P)
    # output row pairs (2k, 2k+1) stored with one contiguous-in-DRAM DMA
    out_r = out[:].rearrange("(u v) (p c) -> u p v c", v=2, p=P)

    U = T // 2
    with TileContext(nc) as tc:
        with (
            tc.tile_pool(name="ds", bufs=2) as pds,
            tc.tile_pool(name="db", bufs=min(U, 5)) as pdb,
            tc.tile_pool(name="q", bufs=U) as pq,
            tc.tile_pool(name="sr", bufs=1) as psr,
        ):

            def body():
                # No tile is ever recycled within a rep (db/q bufs=U): a
                # recycled slot's release joins waits from several engines —
                # multi-waits the codegen only tolerates via legalization
                # nops; plenty of SBUF, so avoid them outright.
                #
                # All loads are pre-issued so each queue's program order is
                # loads-then-stores: Pool = SWDGE cast-loads of even pairs
                # (u8 -> bf16 in the DMA datapath) then cast-stores; SP =
                # u8 loads of odd pairs; ACT = converts of odd pairs. The
                # two convert paths alternate pair-for-pair so bf16 data is
                # produced in exactly the order DVE consumes it.
                sru = psr.tile([P, nsr * C], u8, tag="sru")
                nc.sync.dma_start(
                    sru[:].rearrange("p (k c) -> p k c", c=C), sri_r
                )
                # sr convert on DVE (2x_2p copy, ~2 us): keeps ACT free to
                # start pair converts immediately and un-gates DVE's ramp
                srt = psr.tile([P, nsr * C], bf, tag="srb")
                nc.vector.tensor_copy(srt[:], sru[:])
                sr_slice = [srt[:, k * C : (k + 1) * C] for k in range(nsr)]

                db_tiles = {}
                for u in range(U):
                    db = pdb.tile([P, 2 * C], bf)
                    if u % 2 == 0:
                        nc.gpsimd.dma_start(db[:], bits_r[u])
                    else:
                        ds = pds.tile([P, 2 * C], u8)
                        nc.sync.dma_start(ds[:], bits_r[u])
                        nc.scalar.copy(db[:], ds[:])
                    db_tiles[u] = db

                q_slot = {}  # t -> AP of its [P, C] half
                for t in range(T):
                    u, v = divmod(t, 2)
                    if v == 0:
                        pair = pq.tile([P, 2 * C], bf)
                        q_slot[t] = pair[:, 0:C]
                        q_slot[t + 1] = pair[:, C : 2 * C]
                    db = db_tiles[u]
                    w_t = db[:, v * C : (v + 1) * C]

                    # single-op select: q = is_ge(hq, w), w host-packed as
                    # 1 (dvs=0: pass hq), 0 (dvs=1,dvd=1: emit 1),
                    # 2 (dvs=1,dvd=0: emit 0) — exact on bits
                    qt = q_slot[t]
                    kind, idx = sched[t]
                    src = q_slot[idx] if kind == "q" else sr_slice[idx]
                    nc.vector.tensor_tensor(
                        qt, src, w_t, mybir.AluOpType.is_ge
                    )
                    if v == 1:
                        # SWDGE cast-store: bf16 in SBUF -> u8 in HBM
                        nc.gpsimd.dma_start(
                            out_r[u],
                            pair[:].rearrange("p (v c) -> p v c", c=C),
                        )

            if reps == 1:
                body()
            else:
                with tc.For_i(0, reps, 1):
                    body()
    return _legalize_waits(nc) if legalize else nc


def kernel(dividend, divisor, sr_init, rng_table):
    global LAST_RESULTS
    rng_host = np.asarray(rng_table).astype(np.int64)

    dividend = np.asarray(dividend)
    divisor = np.asarray(divisor)
    T, N = dividend.shape
    buf_dep = np.asarray(sr_init).shape[0]
    assert N % (N_CORES * P) == 0, N
    NS = N // N_CORES

    sched, sr_rows = _schedule(T, buf_dep, rng_host)
    n_sr = len(sr_rows)
    key = (T, NS, sched, n_sr, REPS)
    nc = _nc_cache.get(key)
    if nc is None:
        nc = _build(T, NS, sched, n_sr, reps=REPS)
        _nc_cache[key] = nc

    # bits {0,1}: device only ever needs divisor and m = dividend*divisor
    # (q = max(hq - divisor, m)), so precompute m here and pack both as
    # uint8, pre-arranged into the on-chip tile layout [u][p][v,s,c] so
    # each pair-load is contiguous
    C = NS // P
    dvs_u8 = np.asarray(divisor).astype(np.uint8)
    dvd_u8 = np.asarray(dividend).astype(np.uint8)
    w = (1 - dvs_u8) + 2 * (dvs_u8 * (1 - dvd_u8))  # {1, 0, 2} selector
    bits = w.reshape(T // 2, 2, N_CORES, P, C)  # u,v,core,p,c
    bits = bits.transpose(2, 0, 3, 1, 4)  # core,u,p,v,c
    sr_np = np.asarray(sr_init)
    sr_used = (
        sr_np[sr_rows].astype(np.uint8)
        if n_sr
        else np.zeros((1, N), np.uint8)
    )
    in_maps = []
    for c in range(N_CORES):
        sl = slice(c * NS, (c + 1) * NS)
        in_maps.append(
            {
                "bits": np.ascontiguousarray(bits[c]).reshape(T // 2, P, 2 * C),
                "sr_init": np.ascontiguousarray(sr_used[:, sl]),
            }
        )

    res = run_bass_kernel_spmd(nc, in_maps, core_ids=list(range(N_CORES)))
    LAST_RESULTS = res
    out = np.concatenate([m["quotient"] for m in res.results], axis=1)
    return out.astype(np.float32)  # u8 {0,1} -> f32, exact



# revision 3
# speedup vs baseline: 5.6268x; 5.6268x over previous
"""CORDIV stochastic-computing division kernel for Trainium2 (8 NeuronCores).

Every stream in this problem is a bitstream ({0,1}), so the host packs 32
lanes into one uint32 word and the device runs the whole recurrence with
bitwise ops:

    q[t] = dvs[t] ? dvd[t] : hq          (per lane)
         = (hq AND a[t]) OR b[t]         (bitwise, 32 lanes/word, exact)
    a[t] = ~dvs[t]  (pass-through mask), b[t] = dvd[t] AND dvs[t]

where hq = q[t-1-r_t] (or a packed sr_init row for t-1-r_t < 0); the tiny
rng_table gather schedule is resolved on the host, so the device kernel is
a static DAG of bitwise tensor_tensor ops on [128, n*W] u32 tiles
(W = N/(cores*128*32) words/partition/step).

Structure (all of it measured on this stack, see docstrings below):
  * Steps live in SBUF in dependency-level order, each level's steps
    ordered by their source's position in the previous level. All sources
    of one level are then contiguous, so the ANDs of a level merge into
    1-2 wide DVE ops and the ORs into exactly one — 13 DVE instructions
    total instead of 32 (per-instruction overhead dominates at this tile
    size). sr_init rows are duplicated host-side so level 1 is one op too,
    and ride in the same DRAM tensor as the selector planes.
  * 2 loads + 2 stores per body, split across the two HWDGE queues
    (SP/ACT) with ~equal bytes. gpsimd/SWDGE is avoided entirely (breaks
    under For_i on this runtime).
  * Tile's For_i inserts a full all-engine barrier + DMA-sem rewind every
    iteration (no cross-iteration overlap), so for REPS>1 the body is
    software-pipelined INNER times inside one iteration with 2 bodies of
    load lookahead: loads never sit behind stores in queue program order.

HBM traffic per core per rep is ~1.56 MiB (1.07 MiB packed selector
planes + sr in, 0.5 MiB packed quotient out) vs ~8 MiB for the u8
baseline; measured steady-state ~6.6 us/rep across 8 cores ≈ the shared
HBM roofline (~290 GB/s/core), down from the 51 us baseline.

Sharding: lane dimension N split evenly across 8 cores (data parallel,
no communication).
"""

import numpy as np

import concourse.bass as bass
import concourse.mybir as mybir
from concourse.tile import TileContext
from concourse.bass_utils import run_bass_kernel_spmd

N_CORES = 8
P = 128  # SBUF partitions
LW = 32  # lanes per u32 word

_nc_cache: dict = {}
LAST_RESULTS = None  # test harness introspection
REPS = 1  # >1: HW-loop reps (timing harness only; output unchanged)
INNER = 12  # software-pipelined bodies per For_i iteration when REPS>1
LOOKAHEAD = 2  # bodies of load lookahead in the software pipeline
COARSE = True  # 2 loads + 2 stores per body (vs per-level chunks)


def _schedule(T, buf_dep, rng_table):
    """Host-side resolution of the shift-register gather into a static DAG.

    Returns (sched, sr_rows): sched[t] = ("q", j) meaning src is quotient row
    j, or ("s", k) meaning src is the k-th entry of sr_rows (a compacted list
    of the sr_init rows actually referenced).
    """
    rng = [int(rng_table[t % buf_dep]) for t in range(T)]
    sched = []
    for t in range(T):
        r = rng[t]
        j = t - 1 - r
        if j >= 0:
            sched.append(("q", j))
        else:
            sched.append(("s", r - t))
    sr_rows = sorted({k for kind, k in sched if kind == "s"})
    row_pos = {k: i for i, k in enumerate(sr_rows)}
    sched = [(kind, k if kind == "q" else row_pos[k]) for kind, k in sched]
    return tuple(sched), sr_rows


def _layout(sched):
    """Level-ordered step layout + merged-op plan.

    Returns (order, pos, levels, and_groups): `order` is the step id at
    each layout position (level-major; within a level sorted by the
    source's layout position so consecutive steps have consecutive
    sources); and_groups is a list of (steps_run, src_kind,
    src_first_pos), each run being one DVE AND op. Level-1 runs
    reference duplicated sr rows (kind "s") shipped in run order.
    """
    T = len(sched)
    depth = [0] * T
    for t in range(T):
        kind, j = sched[t]
        depth[t] = 1 if kind == "s" else depth[j] + 1
    levels = []
    for d in range(1, max(depth) + 1):
        levels.append([t for t in range(T) if depth[t] == d])

    order = []
    pos = {}
    and_groups = []
    for li, lv in enumerate(levels):
        if li == 0:
            lv_sorted = sorted(lv)
        else:
            lv_sorted = sorted(lv, key=lambda t: (pos[sched[t][1]], t))
        for t in lv_sorted:
            pos[t] = len(order)
            order.append(t)
        if li == 0:
            and_groups.append((lv_sorted, "s", 0))
        else:
            run = [lv_sorted[0]]
            for t in lv_sorted[1:]:
                if pos[sched[t][1]] == pos[sched[run[-1]][1]] + 1:
                    run.append(t)
                else:
                    and_groups.append((run, "q", pos[sched[run[0]][1]]))
                    run = [t]
            and_groups.append((run, "q", pos[sched[run[0]][1]]))
        levels[li] = lv_sorted
    return order, pos, levels, and_groups


def _legalize_waits(nc):
    """Make the emitted BIR digestible by this walrus build.

    codegen accepts at most ONE sync wait per instruction (any opcode,
    Drain included). Extra waits are hoisted onto preceding same-engine
    NoOps — engines execute their streams in order, so blocking
    semantics are identical. (InstIncSwdgeSem rewriting kept for safety
    but unused: no SWDGE queues here.)
    """
    n = 0
    for blk in nc.m.functions[0].blocks:
        new_insts = []
        for inst in blk.instructions:
            if type(inst).__name__ == "InstIncSwdgeSem":
                if inst._mode == "add":
                    continue
                assert inst._mode == "sub", inst._mode
                for i, (val, name) in enumerate(
                    zip(inst._sem_values, inst._sem_names)
                ):
                    if val == 0:
                        continue
                    upd = mybir.SyncUpdate(
                        sync_type="semaphore",
                        id=inst._sem_id_base + i,
                        update_mode="sem-sub-imm",
                        update_value=val,
                        ant_name=name,
                    )
                    new_insts.append(
                        mybir.InstNoOp(
                            name=f"{inst.name}_swdgesem_{n}",
                            engine=inst.engine,
                            ins=[],
                            outs=[],
                            sync_info=mybir.SyncInfo(
                                on_wait=[], on_update=[upd]
                            ),
                        )
                    )
                    n += 1
            else:
                new_insts.append(inst)
        blk.instructions = new_insts
    for blk in nc.m.functions[0].blocks:
        new_insts = []
        for inst in blk.instructions:
            si = inst.sync_info
            waits = list(si.on_wait) if si is not None and si.on_wait is not None else []
            if len(waits) > 1 and inst.opcode != "ISA":
                for w in waits[:-1]:
                    nop = mybir.InstNoOp(
                        name=f"{inst.name}_waitnop_{n}",
                        engine=inst.engine,
                        ins=[],
                        outs=[],
                        sync_info=mybir.SyncInfo(on_wait=[w], on_update=[]),
                    )
                    new_insts.append(nop)
                    n += 1
                inst.sync_info = mybir.SyncInfo(
                    on_wait=[waits[-1]], on_update=list(si.on_update or [])
                )
            new_insts.append(inst)
        blk.instructions = new_insts
    return nc


def _build(
    T,
    NS,
    sched,
    reps=1,
    inner=None,
    legalize=True,
    lookahead=None,
    coarse=None,
):
    """Emit the per-core Bass/Tile module. NS = lanes per core."""
    if inner is None:
        inner = INNER
    if lookahead is None:
        lookahead = LOOKAHEAD
    if coarse is None:
        coarse = COARSE
    # the For_i iteration barrier precludes cross-iteration overlap, so
    # multi-buffering would only waste SBUF in the looped timing builds
    bufs = 2 if reps == 1 else 1
    NSW = NS // LW  # u32 words per step per core
    W = NSW // P  # words per partition per step
    assert NSW % P == 0
    u32 = mybir.dt.uint32
    order, pos, levels, and_groups = _layout(sched)
    n_sr = len(levels[0])  # duplicated sr rows, one per level-1 step
    n_lv = len(levels)

    nc = bass.Bass()
    # single input tensor: [sr rows | per level: a-block, b-block]
    bits = nc.dram_tensor(
        "bits", [P, (n_sr + T * 2) * W], u32, kind="ExternalInput"
    )
    out = nc.dram_tensor("quotient", [P, T * W], u32, kind="ExternalOutput")

    AND = mybir.AluOpType.bitwise_and
    OR = mybir.AluOpType.bitwise_or

    # bits offsets: sr block first, then per level a-block/b-block
    a_off = {}
    b_off = {}
    off = n_sr * W
    lvl_bound = [0]  # chunk boundaries INCLUDE the sr block in chunk 0
    for lv in levels:
        for i, t in enumerate(lv):
            a_off[t] = off + i * W
            b_off[t] = off + (len(lv) + i) * W
        off += 2 * len(lv) * W
        lvl_bound.append(off)
    assert off == (n_sr + T * 2) * W

    # queue plans: (engine_name, [level_indices]) for bits loads (chunk 0
    # includes the sr block). Stores are (engine, pos_range), issued after
    # all covering levels complete. Contiguous level runs coalesce into a
    # single DMA, so coarse mode is 2 loads + 2 stores per body.
    lv_start = [sum(len(x) for x in levels[:i]) for i in range(n_lv + 1)]
    if n_lv >= 5:
        if coarse:
            load_plan = [("sp", [0, 1]), ("act", [2, 3, 4])]
            store_plan = [
                ("sp", (lv_start[0], lv_start[3])),
                ("act", (lv_start[3], lv_start[5])),
            ]
        else:
            load_plan = [
                ("sp", [0]),
                ("act", [1]),
                ("sp", [2]),
                ("act", [3]),
                ("sp", [4]),
            ]
            store_plan = [
                ("act", (lv_start[0], lv_start[2])),
                ("sp", (lv_start[2], lv_start[4])),
                ("act", (lv_start[4], lv_start[5])),
            ]
    else:
        load_plan = [("sp", list(range(n_lv)))]
        store_plan = [("act", (0, lv_start[n_lv]))]

    with TileContext(nc) as tc:
        with (
            tc.tile_pool(name="in", bufs=bufs) as pin,
            tc.tile_pool(name="q", bufs=bufs) as pq,
        ):
            eng = {"sp": nc.sync, "act": nc.scalar}

            def load_part(tag):
                bt = pin.tile([P, (n_sr + T * 2) * W], u32, tag=f"bits{tag}")
                for e, lis in load_plan:
                    c0 = lvl_bound[lis[0]]
                    c1 = lvl_bound[lis[-1] + 1]
                    eng[e].dma_start(bt[:, c0:c1], bits[:, c0:c1])
                return bt

            def compute_part(tag, bt):
                qt = pq.tile([P, T * W], u32, tag=f"q{tag}")
                for li, lv in enumerate(levels):
                    lv_set = set(lv)
                    for run, kind, src0 in [
                        g for g in and_groups if g[0][0] in lv_set
                    ]:
                        n = len(run)
                        p0 = pos[run[0]]
                        dst = qt[:, p0 * W : (p0 + n) * W]
                        if kind == "s":
                            # sr rows are the first n_sr*W words of bt
                            src = bt[:, 0 : n * W]
                        else:
                            src = qt[:, src0 * W : (src0 + n) * W]
                        am = bt[:, a_off[run[0]] : a_off[run[0]] + n * W]
                        nc.vector.tensor_tensor(dst, src, am, AND)
                    p0 = lv_start[li]
                    n = len(lv)
                    dst = qt[:, p0 * W : (p0 + n) * W]
                    bm = bt[:, b_off[lv[0]] : b_off[lv[0]] + n * W]
                    nc.vector.tensor_tensor(dst, dst, bm, OR)
                return qt

            def store_part(tag, qt):
                for e, (s0, s1) in store_plan:
                    eng[e].dma_start(
                        out[:, s0 * W : s1 * W], qt[:, s0 * W : s1 * W]
                    )

            def pipelined(n_bodies, tagp):
                # software pipeline: body k+lookahead's loads are emitted
                # (and so sit in each DMA queue) before body k's stores
                la = min(lookahead, n_bodies)
                loaded = [load_part(f"{tagp}{i}") for i in range(la)]
                for k in range(n_bodies):
                    if k + la < n_bodies:
                        loaded.append(load_part(f"{tagp}{k + la}"))
                    qt = compute_part(f"{tagp}{k}", loaded[k])
                    store_part(f"{tagp}{k}", qt)

            if reps == 1:
                bt = load_part("0")
                qt = compute_part("0", bt)
                store_part("0", qt)
            else:
                trips, rem = divmod(reps, inner)
                if trips > 0:
                    with tc.For_i(0, trips, 1):
                        pipelined(inner, "L")
                if rem:
                    # reuse the loop bodies' tags: no extra SBUF footprint
                    pipelined(rem, "L")
    return _legalize_waits(nc) if legalize else nc


def kernel(dividend, divisor, sr_init, rng_table):
    global LAST_RESULTS
    rng_host = np.asarray(rng_table).astype(np.int64)

    dividend = np.asarray(dividend)
    divisor = np.asarray(divisor)
    T, N = dividend.shape
    buf_dep = np.asarray(sr_init).shape[0]
    assert N % (N_CORES * P * LW) == 0, N
    NS = N // N_CORES
    NSW = NS // LW
    W = NSW // P

    sched, sr_rows = _schedule(T, buf_dep, rng_host)
    order, pos, levels, and_groups = _layout(sched)
    key = (T, NS, sched, REPS, INNER, LOOKAHEAD, COARSE)
    nc = _nc_cache.get(key)
    if nc is None:
        nc = _build(T, NS, sched, reps=REPS)
        _nc_cache[key] = nc

    # bitstreams -> packed bitplanes: a = pass-through mask (~dvs),
    # b = emitted value (dvd & dvs); q = (hq & a) | b  exactly
    dvs = divisor.astype(np.uint8)
    dvd = dividend.astype(np.uint8)
    a32 = np.packbits(1 - dvs, axis=1).view(np.uint32)  # [T, N/32]
    b32 = np.packbits(dvd & dvs, axis=1).view(np.uint32)
    # duplicated sr rows: one per level-1 step, in level-1 layout order
    sr_np = np.asarray(sr_init)
    sr_sel = np.stack(
        [sr_np[sr_rows[sched[t][1]]] for t in levels[0]]
    ).astype(np.uint8)
    sr32 = np.packbits(sr_sel, axis=1).view(np.uint32)  # [n_sr, N/32]
    n_sr = len(levels[0])

    # [sr rows | per level: a-block rows then b-block rows]
    plane_rows = []
    for lv in levels:
        plane_rows += [("a", t) for t in lv] + [("b", t) for t in lv]
    stacked = np.concatenate(
        [sr32, np.stack([(a32 if p == "a" else b32)[t] for p, t in plane_rows])]
    )  # [n_sr + 2T, N/32]

    in_maps = []
    nrow = n_sr + 2 * T
    for c in range(N_CORES):
        sl = slice(c * NSW, (c + 1) * NSW)
        bits_c = np.ascontiguousarray(
            stacked[:, sl].reshape(nrow, P, W).transpose(1, 0, 2)
        ).reshape(P, nrow * W)
        in_maps.append({"bits": bits_c})

    res = run_bass_kernel_spmd(nc, in_maps, core_ids=list(range(N_CORES)))
    LAST_RESULTS = res
    qw = np.concatenate(
        [
            m["quotient"].reshape(P, T, W).transpose(1, 0, 2).reshape(T, NSW)
            for m in res.results
        ],
        axis=1,
    )  # [T(layout order), N/32] u32
    inv = np.array([pos[t] for t in range(T)])
    qw = qw[inv]  # back to step order
    q = np.unpackbits(np.ascontiguousarray(qw).view(np.uint8), axis=1)
    return q.astype(np.float32)
